# revision 62
# baseline (speedup 1.0000x reference)
"""Trainium2 Bass kernel for nn_CrossAttention_61735859912918.

B=4, SQ=SKV=2048, D=768, H=12, HD=16 (DR=192).
Sharding: 8 cores = (batch b, SQ half) -> each core computes a [1024, 768]
output slice independently (no collectives; K/V recomputed per SQ-half).

Per-core pipeline (layouts keep the softmax reduction on the matmul
contraction axis, never on partitions):
  - xT/encT loaded via DMA-transpose (bf16, host-cast).
  - QT/KT in "strip" layout: head h of group g occupies partitions
    32*(h%4) .. +17.  Row 16 of each strip is an augmentation row that
    folds the query bias exactly: k_aug = (enc@Wk + bk) @ bq_scaled and
    q_aug = 1, so scores = (q+bq)(k+bk)/sqrt(HD) with no runtime bias adds.
    The value bias bv folds into bp' = bp + bv@Wp host-side (sum(attn)=1).
  - scoresT[kv, q] waves of 2 heads; each wave's two heads go to separate
    PSUM tiles: one gets exact exp on ACT, the other the Schraudolph
    bf16-bitcast exp on DVE (alternating per kv chunk, so every softmax row
    mixes exact/approx kv 50/50).  Separate tiles keep the buffer-release
    chains of the two engines independent.
  - attn@V with [v_h | 1] augmented rhs so the softmax denominator falls
    out of the same matmul; AV matmuls are emitted AV_LAG waves late so the
    tile scheduler's conservative whole-engine barriers get slack.
  - KT/V projections for later kv chunks are emitted inside the qc=0 wave
    stream (PE slack absorbs them); the qc=0 output tail (transpose + Wp)
    is emitted inside the qc=1 wave stream.
"""

import sys

sys.path.insert(0, "/opt/trn_rl_repo")

import numpy as np
import ml_dtypes
from contextlib import ExitStack

import concourse.bass as bass
import concourse.mybir as mybir
import concourse.tile as tile
from concourse import bacc
from concourse.bass import ds, ts
from concourse.masks import make_identity
from concourse import bass_utils

F32 = mybir.dt.float32
BF16 = mybir.dt.bfloat16
I16 = mybir.dt.int16
AF = mybir.ActivationFunctionType
ALU = mybir.AluOpType

B, SQ_FULL, SKV_FULL, D = 4, 2048, 2048, 768
H, HD, DR = 12, 16, 192
P = 128
KO = D // P  # 6
KA = HD + 1  # contraction rows per strip incl. augmentation row

# waves of 2 heads; within each wave the strip indices j=h%4 are distinct so
# the two K=17 score matmuls land on distinct PE row-groups.
WAVES = [
    [(0, 0), (0, 1)],
    [(0, 2), (0, 3)],
    [(1, 0), (1, 1)],
    [(1, 2), (1, 3)],
    [(2, 0), (2, 1)],
    [(2, 2), (2, 3)],
]
NW = len(WAVES)

# Schraudolph bf16 exp: exp(x) ~= bitcast_bf16(int16((2^7/ln2)*x + b)).
# +0.5 compensates truncation; if HW rounds instead, the resulting uniform
# 2^(0.5/128) scale cancels in the softmax ratio.
EXP_A = float(2.0**7 / np.log(2.0))
EXP_B = float(16256.0 - 7.4 + 0.5)

# The attn@V matmuls of wave w are emitted this many waves later, so the
# scheduler's conservative whole-engine barriers get slack instead of
# chaining exp(w) behind AV(w-1) behind exp(w-1).
AV_LAG = 2


def build_program(SQL, SKV):
    """Emit the per-core Bass program. SQL = local q rows, SKV = kv rows."""
    assert SQL % P == 0 and SKV % P == 0
    QCH = min(512, SQL)      # q columns per score wave
    NQC = SQL // QCH
    NS4 = QCH // P           # q subtiles per chunk (4 full-size)
    NK = (NS4 + 1) // 2      # av psum tiles per chunk
    KVC = SKV // P
    NQS = SQL // P
    NKC = SKV // QCH         # kt tiles
    CPK = QCH // P           # kv chunks per kt tile

    nc = bacc.Bacc("TRN2", target_bir_lowering=False, debug=False)

    x_ap = nc.dram_tensor("x", [SQL, D], BF16, kind="ExternalInput").ap()
    enc_ap = nc.dram_tensor("enc", [SKV, D], BF16, kind="ExternalInput").ap()
    wq_ap = nc.dram_tensor("wq_arr", [P, KO, 3, P], BF16, kind="ExternalInput").ap()
    wk_ap = nc.dram_tensor("wk_arr", [P, KO, 3, P], BF16, kind="ExternalInput").ap()
    wv_ap = nc.dram_tensor("wv", [P, KO, DR], BF16, kind="ExternalInput").ap()
    wp0_ap = nc.dram_tensor("wp0", [P, D], BF16, kind="ExternalInput").ap()
    wp1_ap = nc.dram_tensor("wp1", [P, D], BF16, kind="ExternalInput").ap()
    qones_ap = nc.dram_tensor("qones", [P, 1], F32, kind="ExternalInput").ap()
    bk_ap = nc.dram_tensor("bk_strip", [3, P, 1], F32, kind="ExternalInput").ap()
    bp_ap = nc.dram_tensor("bp_row", [1, D], BF16, kind="ExternalInput").ap()
    out_ap = nc.dram_tensor("out", [SQL, D], F32, kind="ExternalOutput").ap()

    with TileCtx(nc) as tc, ExitStack() as ctx:
        persist = ctx.enter_context(tc.tile_pool(name="persist", bufs=1))
        esb = ctx.enter_context(tc.tile_pool(name="esb", bufs=6))
        npool = ctx.enter_context(tc.tile_pool(name="npool", bufs=4))
        tailp = ctx.enter_context(tc.tile_pool(name="tailp", bufs=4))
        osbp = ctx.enter_context(tc.tile_pool(name="osbp", bufs=3))

        identb = persist.tile([P, P], BF16, name="identb", tag="identb")
        make_identity(nc, identb)

        # ---- loads, ordered by prologue criticality: tiny bias tiles, then
        # ---- wq + x(qc0), then wk + enc(kc0), then wv, then everything else
        xt_t = [[persist.tile([P, QCH], BF16, name=f"xt{ko}_{qc}", tag=f"xt{ko}_{qc}")
                 for qc in range(NQC)] for ko in range(KO)]
        enct_t = [[persist.tile([P, QCH], BF16, name=f"et{ko}_{kc}", tag=f"et{ko}_{kc}")
                   for kc in range(NKC)] for ko in range(KO)]

        qones_sb = persist.tile([P, 1], F32, name="qones", tag="qones")
        nc.sync.dma_start(qones_sb, qones_ap)
        bk_sb = persist.tile([P, 3, 1], F32, name="bk", tag="bk")
        nc.sync.dma_start(bk_sb, bk_ap.rearrange("g p one -> p g one"))
        wq_sb = persist.tile([P, KO, 3, P], BF16, name="wq", tag="wq")
        nc.sync.dma_start(wq_sb, wq_ap)
        wk_sb = persist.tile([P, KO, 3, P], BF16, name="wk", tag="wk")
        nc.sync.dma_start(wk_sb, wk_ap)
        wv_sb = persist.tile([P, KO, DR], BF16, name="wv", tag="wv")
        nc.sync.dma_start(wv_sb, wv_ap)
        wp0_sb = persist.tile([P, D], BF16, name="wp0", tag="wp0")
        nc.sync.dma_start(wp0_sb, wp0_ap)
        wp1_sb = persist.tile([P, D], BF16, name="wp1", tag="wp1")
        nc.sync.dma_start(wp1_sb, wp1_ap)
        bp_sb = persist.tile([1, D], BF16, name="bp", tag="bp")
        nc.sync.dma_start(bp_sb, bp_ap)
        ones_sb = persist.tile([1, P], BF16, name="ones1", tag="ones1")
        nc.vector.memset(ones_sb, 1.0)
        for ko in range(KO):
            nc.sync.dma_start_transpose(
                xt_t[ko][0], x_ap[ds(0, QCH), ds(ko * P, P)])
        for kc in range(NKC):
            for ko in range(KO):
                nc.sync.dma_start_transpose(
                    enct_t[ko][kc], enc_ap[ds(kc * QCH, QCH), ds(ko * P, P)])
                if kc == 0 and ko == KO - 1:
                    for ko2 in range(KO):
                        for qc in range(1, NQC):
                            nc.sync.dma_start_transpose(
                                xt_t[ko2][qc],
                                x_ap[ds(qc * QCH, QCH), ds(ko2 * P, P)])

        qt_sb = [[persist.tile([P, QCH], BF16, name=f"qt{g}_{qc}", tag=f"qt{g}_{qc}")
                  for qc in range(NQC)] for g in range(3)]
        kt_sb = [[persist.tile([P, QCH], BF16, name=f"kt{g}_{kc}", tag=f"kt{g}_{kc}")
                  for kc in range(NKC)] for g in range(3)]
        va_t = [persist.tile([P, H, 17], BF16, name=f"va{c}", tag=f"va{c}")
                for c in range(KVC)]
        for c in range(KVC):
            nc.vector.memset(va_t[c][:, :, 16:17], 1.0)
        ao_sb = persist.tile([P, NQS, DR], BF16, name="ao", tag="ao")

        # ---- projection emitters: matmuls on PE, strip copies on ACT
        # ---- (Identity + per-partition bias handles bk and the q ones-row)
        def emit_qt(projp, g, qc):
            ps = projp.tile([P, QCH], F32, name="proj", tag="proj")
            for ko in range(KO):
                nc.tensor.matmul(ps, wq_sb[:, ko, g, :], xt_t[ko][qc],
                                 start=(ko == 0), stop=(ko == KO - 1))
            nc.scalar.activation(qt_sb[g][qc], ps, AF.Identity,
                                 bias=qones_sb[:, :])

        def emit_kt(projp, g, kc):
            ps = projp.tile([P, QCH], F32, name="proj", tag="proj")
            for ko in range(KO):
                nc.tensor.matmul(ps, wk_sb[:, ko, g, :], enct_t[ko][kc],
                                 start=(ko == 0), stop=(ko == KO - 1))
            nc.scalar.activation(kt_sb[g][kc], ps, AF.Identity,
                                 bias=bk_sb[:, g, :])

        def emit_v(projp, c):
            ps = projp.tile([P, DR], F32, name="vproj", tag="vproj")
            for ko in range(KO):
                nc.tensor.matmul(
                    ps, enct_t[ko][c * P // QCH][:, ds((c * P) % QCH, P)],
                    wv_sb[:, ko, :], start=(ko == 0), stop=(ko == KO - 1))
            nc.scalar.activation(
                va_t[c][:, :, 0:16],
                ps.rearrange("p (h s) -> p h s", s=16),
                AF.Identity,
            )

        # ---- output tail: DMA-transpose AO[qs] (issued early, right after
        # ---- normalize), then project + store at chunk boundaries.
        # ---- aotx transposes the overlapping window ao[:, 64:192]; its rows
        # ---- 64:128 are ao columns 128:192, matching wp1's base-64 layout.
        def emit_tail_dma(qs):
            aot0 = tailp.tile([P, P], BF16, name="aot0", tag="aot0")
            nc.sync.dma_start_transpose(aot0, ao_sb[:, qs, 0:P])
            aotx = tailp.tile([P, P], BF16, name="aotx", tag="aotx")
            nc.sync.dma_start_transpose(aotx, ao_sb[:, qs, 64:DR])
            return aot0, aotx

        def emit_tail_mm(outp, qs, aot0, aotx):
            for n2 in range(2):
                op = outp.tile([P, 384], F32, name="out", tag="out")
                nc.tensor.matmul(op, aot0, wp0_sb[:, ds(n2 * 384, 384)],
                                 start=True, stop=False)
                nc.tensor.matmul(op, aotx[64:P, :], wp1_sb[64:P, ds(n2 * 384, 384)],
                                 start=False, stop=False)
                # K=1 ones-row matmul adds bp to the same accumulation group
                nc.tensor.matmul(op, ones_sb, bp_sb[:1, ds(n2 * 384, 384)],
                                 start=False, stop=True)
                osb = osbp.tile([P, 384], F32, name=f"osb{n2}", tag=f"osb{n2}")
                if n2 == 0:
                    nc.scalar.activation(osb, op, AF.Identity)
                else:
                    nc.vector.tensor_copy(osb, op)
                nc.sync.dma_start(out_ap[ds(qs * P, P), ds(n2 * 384, 384)], osb)

        # ---- attention wave emitters ----
        def emit_av(avs, pend):
            ea, ed, wave, ia, c, w = pend
            for i, (g, j) in enumerate(wave):
                h = 4 * g + j
                for s4 in range(NS4):
                    k, kk = divmod(s4, 2)
                    if i == ia:
                        lhsT = ea[:, ds(s4 * P, P)]
                    else:
                        lhsT = ed[:, ds(s4 * P, P)].bitcast(BF16)
                    # one accumulation group per av PSUM bank: only the
                    # first mm into the tile may start, only the last
                    # may stop.
                    first = (c == 0 and w == 0 and i == 0 and s4 == 2 * k)
                    last = (c == KVC - 1 and w == NW - 1 and i == 1
                            and s4 == min(2 * k + 1, NS4 - 1))
                    nc.tensor.matmul(
                        avs[k][:, kk, ds(17 * h, 17)],
                        lhsT,
                        va_t[c][:, h, :],
                        start=first, stop=last,
                        skip_group_check=True,
                    )

        def emit_chunk(spsum, avs, qc, c, pending, pre_wave=None):
            ia = c % 2  # which of the wave's two heads gets exact exp
            for w, wave in enumerate(WAVES):
                if pre_wave is not None:
                    pre_wave(w)
                sa = spsum.tile([P, QCH], F32, name="sa", tag="sa")
                sb = spsum.tile([P, QCH], F32, name="sb", tag="sb")
                for i, (g, j) in enumerate(wave):
                    nc.tensor.matmul(
                        sa if i == ia else sb,
                        kt_sb[g][c * P // QCH][32 * j:32 * j + KA,
                            ds((c * P) % QCH, P)],
                        qt_sb[g][qc][32 * j:32 * j + KA, :],
                        start=True, stop=True,
                        tile_position=(32 * j, 0),
                    )
                ea = esb.tile([P, QCH], BF16, name="ea", tag="ea")
                ed = esb.tile([P, QCH], I16, name="ed", tag="ed")
                nc.scalar.activation(ea, sa, AF.Exp)
                nc.vector.tensor_scalar(ed, sb, EXP_A, EXP_B, ALU.mult, ALU.add)
                pending.append((ea, ed, wave, ia, c, w))
                if len(pending) > AV_LAG:
                    emit_av(avs, pending.pop(0))

        def emit_normalize(avs, qc):
            # ao = av[:, :, :16] * (1 / av[:, :, 16])
            for k in range(NK):
                avr = avs[k].rearrange("p kk (h s) -> p kk h s", s=17)
                zr = npool.tile([P, 2, H, 1], F32, name="zr", tag="zr")
                nc.vector.reciprocal(zr[:, :, :, 0], avr[:, :, :, 16])
                for kk in range(2):
                    s4 = 2 * k + kk
                    if s4 >= NS4:
                        break
                    qs = qc * NS4 + s4
                    nc.vector.tensor_tensor(
                        ao_sb[:, qs, :].rearrange("p (h s) -> p h s", s=16),
                        avr[:, kk, :, 0:16],
                        zr[:, kk, :, :].to_broadcast((P, H, 16)),
                        ALU.mult,
                    )

        # ---- phase B: minimal upfront projections (group 0 + va0/va1);
        # ---- the rest are interleaved into the qc=0 wave stream ----
        with tc.tile_pool(name="spsum", bufs=2, space="PSUM") as spsum, \
             tc.tile_pool(name="avpsum", bufs=1, space="PSUM") as avpsum:
            with tc.tile_pool(name="projp", bufs=1, space="PSUM") as projp:
                emit_qt(projp, 0, 0)
                emit_kt(projp, 0, 0)
                emit_v(projp, 0)
                emit_v(projp, 1)

                # ---- phase C: qc=0 attention with interleaved projections ----
                avs0 = [avpsum.tile([P, 2, 204], F32, name=f"av{k}",
                                    tag=f"av{k}") for k in range(NK)]

                def chunk0_prewave(w):
                    # waves 2g/2g+1 use strip group g; emit group g right
                    # before the first wave that needs it
                    if w in (2, 4):
                        emit_qt(projp, w // 2, 0)
                        emit_kt(projp, w // 2, 0)

                pending = []
                for c in range(KVC):
                    if c == 0:
                        emit_chunk(spsum, avs0, 0, c, pending,
                                   pre_wave=chunk0_prewave)
                        emit_v(projp, 2)
                        continue
                    if NQC > 1 and c in (5, 6, 7):
                        emit_qt(projp, c - 5, 1)
                    nxt = c + CPK // 2
                    if nxt < KVC and nxt % CPK == 0:
                        for g in range(3):
                            emit_kt(projp, g, nxt // CPK)
                    if c + 2 < KVC:
                        emit_v(projp, c + 2)
                    emit_chunk(spsum, avs0, 0, c, pending)
                for p in pending:
                    emit_av(avs0, p)
            emit_normalize(avs0, 0)

            # ---- phase D: qc=1 attention with interleaved qc=0 tail ----
            if NQC > 1:
                with tc.tile_pool(name="outp", bufs=2, space="PSUM") as outp:
                    avs1 = [avpsum.tile([P, 2, 204], F32, name=f"av{k}",
                                        tag=f"av{k}") for k in range(NK)]
                    aots = {qs: emit_tail_dma(qs) for qs in range(NS4)}
                    pending = []
                    for c in range(KVC):
                        if c % CPK == CPK - 1 and c // CPK < NS4:
                            qs = c // CPK
                            emit_tail_mm(outp, qs, *aots[qs])
                        emit_chunk(spsum, avs1, 1, c, pending)
                    for p in pending:
                        emit_av(avs1, p)
                    emit_normalize(avs1, 1)

        # ---- phase E: remaining tail (PE transposes: lower latency than
        # ---- DMA transposes for this non-overlapped end-game) ----
        with tc.tile_pool(name="aotp2", bufs=2, space="PSUM") as aotp, \
             tc.tile_pool(name="outp2", bufs=2, space="PSUM") as outp:
            start_qs = NS4 if NQC > 1 else 0
            for qs in range(start_qs, NQS):
                tp = aotp.tile([P, 256], BF16, name="aot", tag="aot")
                nc.tensor.transpose(tp[:, 0:P], ao_sb[:, qs, 0:P], identb)
                nc.tensor.transpose(tp[:64, P:256], ao_sb[:, qs, P:DR], identb)
                aot0 = tailp.tile([P, P], BF16, name="aot0", tag="aot0")
                nc.vector.tensor_copy(aot0, tp[:, 0:P])
                aot1 = tailp.tile([64, P], BF16, name="aot1", tag="aot1")
                nc.vector.tensor_copy(aot1, tp[:64, P:256])
                for n2 in range(2):
                    op = outp.tile([P, 384], F32, name="out", tag="out")
                    nc.tensor.matmul(op, aot0, wp0_sb[:, ds(n2 * 384, 384)],
                                     start=True, stop=False)
                    nc.tensor.matmul(op, aot1,
                                     wp1_sb[0:64, ds(n2 * 384, 384)],
                                     start=False, stop=False)
                    nc.tensor.matmul(op, ones_sb, bp_sb[:1, ds(n2 * 384, 384)],
                                     start=False, stop=True)
                    osb = osbp.tile([P, 384], F32, name=f"osb{n2}",
                                    tag=f"osb{n2}")
                    if n2 == 0:
                        nc.scalar.activation(osb, op, AF.Identity)
                    else:
                        nc.vector.tensor_copy(osb, op)
                    nc.sync.dma_start(out_ap[ds(qs * P, P), ds(n2 * 384, 384)],
                                      osb)

    nc.compile()
    return nc


def TileCtx(nc):
    return tile.TileContext(nc)


def prep_weights(Wq, bq, Wkv, bkv, Wp, bp):
    """Host-side weight prep: strip layouts, bf16 casts, 1/sqrt(HD) and all
    bias folding (bq via the augmentation row, bv via bp' = bp + bv@Wp)."""
    f = np.float32
    Wq = np.asarray(Wq, f)
    Wkv = np.asarray(Wkv, f)
    Wp = np.asarray(Wp, f)
    bq = np.asarray(bq, f)
    bkv = np.asarray(bkv, f)
    bp = np.asarray(bp, f)
    scale = (1.0 / np.sqrt(HD)).astype(f)

    Wq_s = (Wq * scale).reshape(D, H, HD)
    bq_s = (bq * scale).reshape(H, HD)
    Wk = Wkv[:, :DR].reshape(D, H, HD)
    bk = bkv[:DR].reshape(H, HD)
    bv = bkv[DR:]

    # q strips: rows 0..15 = Wq_s, row 16 = 0 (the ones come from the copy
    # bias); k strips: rows 0..15 = Wk, row 16 = Wk @ bq_s per head.
    wq_arr = np.zeros((D, 3, 4, 32), f)
    wk_arr = np.zeros((D, 3, 4, 32), f)
    bk_strip = np.zeros((3, 4, 32), f)
    qones = np.zeros((3, 4, 32), f)
    for h in range(H):
        g, j = divmod(h, 4)
        wq_arr[:, g, j, :16] = Wq_s[:, h]
        wk_arr[:, g, j, :16] = Wk[:, h]
        wk_arr[:, g, j, 16] = Wk[:, h] @ bq_s[h]
        bk_strip[g, j, :16] = bk[h]
        bk_strip[g, j, 16] = bk[h] @ bq_s[h]
        qones[g, j, 16] = 1.0

    return {
        "wq_arr": wq_arr.reshape(KO, P, 3, P).transpose(1, 0, 2, 3).copy()
                        .astype(ml_dtypes.bfloat16),
        "wk_arr": wk_arr.reshape(KO, P, 3, P).transpose(1, 0, 2, 3).copy()
                        .astype(ml_dtypes.bfloat16),
        "wv": Wkv[:, DR:].reshape(KO, P, DR).transpose(1, 0, 2).copy()
                         .astype(ml_dtypes.bfloat16),
        "wp0": Wp[:P].astype(ml_dtypes.bfloat16),
        "wp1": np.concatenate([Wp[P:], Wp[P:]],
                              axis=0).astype(ml_dtypes.bfloat16),
        "qones": qones.reshape(3, P)[0:1].T.copy(),
        "bk_strip": bk_strip.reshape(3, P, 1),
        "bp_row": (bp + bv @ Wp).reshape(1, D).astype(ml_dtypes.bfloat16),
    }


def make_in_maps(hidden_states, encoder_hidden_states, Wq, bq, Wkv, bkv, Wp, bp,
                 n_cores=8):
    """Shard full inputs into per-core in_maps. core i -> (b=i//2, half=i%2)."""
    hs = np.asarray(hidden_states, np.float32)
    enc = np.asarray(encoder_hidden_states, np.float32)
    w = prep_weights(Wq, bq, Wkv, bkv, Wp, bp)
    sql = SQ_FULL // 2
    in_maps = []
    for i in range(n_cores):
        b, half = divmod(i, 2)
        m = dict(w)
        m["x"] = hs[b, half * sql:(half + 1) * sql].astype(ml_dtypes.bfloat16)
        m["enc"] = enc[b].astype(ml_dtypes.bfloat16)
        in_maps.append(m)
    return in_maps


_PROGRAM_CACHE = {}


def get_program(SQL=SQ_FULL // 2, SKV=SKV_FULL):
    key = (SQL, SKV)
    if key not in _PROGRAM_CACHE:
        _PROGRAM_CACHE[key] = build_program(SQL, SKV)
    return _PROGRAM_CACHE[key]


def kernel(hidden_states, encoder_hidden_states, Wq, bq, Wkv, bkv, Wp, bp,
           **run_kwargs):
    nc = get_program()
    in_maps = make_in_maps(hidden_states, encoder_hidden_states,
                           Wq, bq, Wkv, bkv, Wp, bp)
    res = bass_utils.run_bass_kernel_spmd(nc, in_maps, core_ids=list(range(8)),
                                          **run_kwargs)
    sql = SQ_FULL // 2
    out = np.empty((B, SQ_FULL, D), np.float32)
    for i in range(8):
        b, half = divmod(i, 2)
        out[b, half * sql:(half + 1) * sql] = res.results[i]["out"]
    if run_kwargs:
        kernel.last_results = res
    return out
